# revision 4
# baseline (speedup 1.0000x reference)
"""Trainium2 Bass kernel for nn_MultiHeadAttention (B=2, S=2048, D=1024, H=16).

Sharding: 8 cores; core c handles batch b=c//4 and the 4 heads
h in [4*(c%4), 4*(c%4)+4). Attention is embarrassingly parallel over (B, H);
the output projection is computed per-core over its head group (partial sums),
and the host sums the 4 partials per batch and adds the output bias.

Per-core dataflow (all fp32; contraction dim always on SBUF partitions):
  - host pre-transposes q/k/v per batch -> qT/kT/vT [D, S] (layout prep only)
  - qh^T / kh^T [d, s] computed 2-heads-packed: head A on partitions 0-63,
    head B on 64-127 (lhsT = packed Wq columns, rhs = streamed xT chunks)
  - vh computed in natural [s, d] layout (lhsT = vT chunk, rhs = Wv columns),
    with a ones-column appended -> AV matmul also yields softmax denominators
  - scores computed transposed s^T[k, q] so the softmax numerator
    exp(0.125*s + log2*causal) is produced by ScalarE directly in the
    AV-ready layout (k on partitions); no transposes needed anywhere on-chip.
    The reference's "mask" log(tril*1e-9 + 1e-9) is, by softmax shift
    invariance, exactly a x2 weight on the lower triangle.
  - the two heads of a pack issue score matmuls from partition bases 0/64,
    which the PE runs concurrently (row tiling)
  - AV: psum[65, 512] accumulates vh_aug.T @ e^T over 16 k-chunks; row 64 is
    the denominator. Normalize via DVE reciprocal + K=1 broadcast matmul.
  - out projection: per s-block, 4 head matmuls (K=64) accumulate in psum;
    partial [S, D] DMAed out.
"""
import numpy as np
from contextlib import ExitStack

import concourse.bacc as bacc
import concourse.mybir as mybir
import concourse.tile as tile
from concourse.bass_utils import run_bass_kernel_spmd

F32 = mybir.dt.float32
AF = mybir.ActivationFunctionType
ALU = mybir.AluOpType

B, S, D, H, PD = 2, 2048, 1024, 16, 64
NCORES = 8
HPC = H * B // NCORES        # 4 heads per core
NPACK = HPC // 2             # 2 head-pairs per core
SC = 512                     # free-dim chunk (one fp32 psum bank)
NSC = S // SC                # 4
NKB = S // 128               # 16 key blocks / s blocks
NDC = D // 128               # 8 contraction chunks for the projections
LOG2 = float(np.log(2.0))

# cst blob column layout (per partition)
CST_MASK = 0                 # [4, 512] diagonal-block multipliers
CST_BQ = CST_MASK + 4 * SC   # [2] per-pack bq (per-partition scalars)
CST_BK = CST_BQ + 2          # [2]
CST_BV = CST_BK + 2          # [256] bv broadcast (free-dim layout)
CST_LOG2 = CST_BV + HPC * PD # [1] log(2) per partition (exp bias)
CST_ZERO = CST_LOG2 + 1      # [1] 0.0 per partition (exp bias)
CST_COLS = CST_ZERO + 1


def _build(causal: bool):
    nc = bacc.Bacc()
    qT = nc.dram_tensor("qT", [D, S], F32, kind="ExternalInput")
    kT = nc.dram_tensor("kT", [D, S], F32, kind="ExternalInput")
    vT = nc.dram_tensor("vT", [D, S], F32, kind="ExternalInput")
    wq = nc.dram_tensor("wq", [D, HPC * PD], F32, kind="ExternalInput")
    wk = nc.dram_tensor("wk", [D, HPC * PD], F32, kind="ExternalInput")
    wv = nc.dram_tensor("wv", [D, HPC * PD], F32, kind="ExternalInput")
    wo = nc.dram_tensor("wo", [HPC * PD, D], F32, kind="ExternalInput")
    cst = nc.dram_tensor("cst", [128, CST_COLS], F32, kind="ExternalInput")
    out_d = nc.dram_tensor("out", [S, D], F32, kind="ExternalOutput")

    with tile.TileContext(nc) as tc, ExitStack() as ctx:
        cpool = ctx.enter_context(tc.tile_pool(name="cpool", bufs=1))
        xpool = ctx.enter_context(tc.tile_pool(name="xpool", bufs=2))
        hpool = ctx.enter_context(tc.tile_pool(name="hpool", bufs=1))
        epool = ctx.enter_context(tc.tile_pool(name="epool", bufs=3))
        opool = ctx.enter_context(tc.tile_pool(name="opool", bufs=2))
        spool = ctx.enter_context(tc.tile_pool(name="spool", bufs=2))
        pspool = ctx.enter_context(tc.tile_pool(name="ps", bufs=2, space="PSUM"))

        # ---- constants ----
        cst_t = cpool.tile([128, CST_COLS], F32)
        nc.sync.dma_start(cst_t[:], cst[:])
        wq_t = cpool.tile([128, NDC, HPC * PD], F32)
        nc.sync.dma_start(wq_t[:], wq[:].rearrange("(c p) m -> p c m", p=128))
        wk_t = cpool.tile([128, NDC, HPC * PD], F32)
        nc.sync.dma_start(wk_t[:], wk[:].rearrange("(c p) m -> p c m", p=128))
        wv_t = cpool.tile([128, NDC, HPC * PD], F32)
        nc.sync.dma_start(wv_t[:], wv[:].rearrange("(c p) m -> p c m", p=128))
        wo_t = cpool.tile([PD, HPC, D], F32)
        nc.sync.dma_start(wo_t[:], wo[:].rearrange("(h p) n -> p h n", p=PD))
        ones1 = cpool.tile([1, PD], F32)
        nc.vector.memset(ones1[:], 1.0)

        def mask_ap(delta):
            return cst_t[:, CST_MASK + delta * SC: CST_MASK + (delta + 1) * SC]

        # ---- q/k projections, ^T layout, 2 heads packed ----
        qh = [hpool.tile([128, S], F32, name=f"qh{p}") for p in range(NPACK)]
        kh = [hpool.tile([128, S], F32, name=f"kh{p}") for p in range(NPACK)]
        for xdram, wtile, htiles, boff in (
            (qT, wq_t, qh, CST_BQ),
            (kT, wk_t, kh, CST_BK),
        ):
            for sc in range(NSC):
                xTc = xpool.tile([128, NDC, SC], F32, tag="xTc", name="xTc")
                nc.sync.dma_start(
                    xTc[:],
                    xdram[:, sc * SC:(sc + 1) * SC].rearrange(
                        "(c p) s -> p c s", p=128),
                )
                for pk in range(NPACK):
                    ps = pspool.tile([128, SC], F32, tag="mm", name="ps_qk")
                    for dc in range(NDC):
                        nc.tensor.matmul(
                            ps[:],
                            wtile[:, dc, pk * 128:(pk + 1) * 128],
                            xTc[:, dc, :],
                            start=(dc == 0), stop=(dc == NDC - 1),
                        )
                    # evacuate + bias (per-partition scalar = per-d)
                    nc.scalar.activation(
                        htiles[pk][:, sc * SC:(sc + 1) * SC], ps[:],
                        AF.Identity,
                        bias=cst_t[:, boff + pk: boff + pk + 1],
                    )

        # ---- v projection, natural [s, d] layout, 4 heads side by side ----
        vh_all = hpool.tile([128, NKB, HPC, PD + 1], F32, name="vh_all")
        nc.vector.memset(vh_all[:, :, :, PD:PD + 1], 1.0)
        bv_ap = cst_t[:, CST_BV: CST_BV + HPC * PD].rearrange(
            "p (h d) -> p h d", h=HPC)
        for sb in range(NKB):
            vsl = xpool.tile([128, NDC, 128], F32, tag="vsl", name="vsl")
            nc.sync.dma_start(
                vsl[:],
                vT[:, sb * 128:(sb + 1) * 128].rearrange(
                    "(c p) j -> p c j", p=128),
            )
            ps = pspool.tile([128, HPC * PD], F32, tag="mm", name="ps_v")
            for dc in range(NDC):
                nc.tensor.matmul(
                    ps[:], vsl[:, dc, :], wv_t[:, dc, :],
                    start=(dc == 0), stop=(dc == NDC - 1),
                )
            nc.vector.tensor_tensor(
                vh_all[:, sb, :, 0:PD],
                ps[:].rearrange("p (h d) -> p h d", h=HPC),
                bv_ap,
                ALU.add,
            )

        # ---- attention (per qc x pack), then out-projection per qc ----
        for qc in range(NSC):
            ohs = []  # per-head normalized outh^T [64, SC] for this qc
            for pk in range(NPACK):
                avs = [
                    pspool.tile([PD + 1, SC], F32, tag="av", name=f"av{hh}")
                    for hh in range(2)
                ]
                prev = None
                for kb in range(NKB):
                    cur = []
                    for hh in range(2):
                        base = hh * PD
                        sps = pspool.tile([128, SC], F32, tag=f"s{hh}",
                                          name=f"sps{hh}")
                        nc.tensor.matmul(
                            sps[:],
                            kh[pk][base:base + PD, kb * 128:(kb + 1) * 128],
                            qh[pk][base:base + PD, qc * SC:(qc + 1) * SC],
                        )
                        et = epool.tile([128, SC], F32, tag=f"e{hh}",
                                        name=f"et{hh}")
                        delta = kb - 4 * qc
                        boff = CST_LOG2 if (causal and delta < 0) else CST_ZERO
                        nc.scalar.activation(
                            et[:], sps[:], AF.Exp,
                            bias=cst_t[:, boff:boff + 1], scale=0.125)
                        if causal and 0 <= delta < 4:
                            nc.vector.tensor_tensor(
                                et[:], et[:], mask_ap(delta), ALU.mult)
                        cur.append(et)
                    if prev is not None:
                        for hh in range(2):
                            nc.tensor.matmul(
                                avs[hh][:],
                                vh_all[:, kb - 1, pk * 2 + hh, :],
                                prev[hh][:],
                                start=(kb - 1 == 0), stop=False,
                            )
                    prev = cur
                for hh in range(2):
                    nc.tensor.matmul(
                        avs[hh][:],
                        vh_all[:, NKB - 1, pk * 2 + hh, :],
                        prev[hh][:],
                        start=False, stop=True,
                    )
                # normalize: outh^T = av[0:64] * bcast(1/av[64])
                for hh in range(2):
                    av = avs[hh]
                    drow = spool.tile([1, SC], F32, tag="drow", name="drow")
                    nc.vector.tensor_copy(drow[:], av[PD:PD + 1, :])
                    rrow = spool.tile([1, SC], F32, tag="rrow", name="rrow")
                    nc.vector.reciprocal(rrow[:], drow[:])
                    bps = pspool.tile([PD, SC], F32, tag="mm", name="bps")
                    nc.tensor.matmul(bps[:], ones1[:], rrow[:])
                    rb = spool.tile([PD, SC], F32, tag="rb", name="rb")
                    nc.vector.tensor_copy(rb[:], bps[:])
                    oh = opool.tile([PD, SC], F32, tag=f"oh{pk * 2 + hh}",
                                    name=f"oh{pk * 2 + hh}")
                    nc.vector.tensor_tensor(oh[:], av[0:PD, :], rb[:],
                                            ALU.mult)
                    ohs.append(oh)
            # out projection for s rows covered by this qc
            for sbl in range(4):
                sb = qc * 4 + sbl
                for dc2 in range(2):
                    pps = pspool.tile([128, SC], F32, tag="mm", name="pps")
                    for h in range(HPC):
                        nc.tensor.matmul(
                            pps[:],
                            ohs[h][:, sbl * 128:(sbl + 1) * 128],
                            wo_t[:, h, dc2 * SC:(dc2 + 1) * SC],
                            start=(h == 0), stop=(h == HPC - 1),
                        )
                    oev = opool.tile([128, SC], F32, tag="oev", name="oev",
                                     bufs=3)
                    nc.scalar.copy(oev[:], pps[:])
                    nc.sync.dma_start(
                        out_d[sb * 128:(sb + 1) * 128,
                              dc2 * SC:(dc2 + 1) * SC],
                        oev[:],
                    )

    nc.compile()
    return nc


_programs = {}


def _get_program(causal: bool):
    if causal not in _programs:
        _programs[causal] = _build(causal)
    return _programs[causal]


def _make_cst(bq4, bk4, bv4, causal: bool) -> np.ndarray:
    """Per-core constant blob [128, CST_COLS]."""
    cst = np.zeros((128, CST_COLS), np.float32)
    # diagonal-block multipliers: mask_delta[k_local, q_local] = 2 iff
    # q_local - 128*delta >= k_local (else 1); all-ones when not causal
    for delta in range(4):
        if causal:
            kloc = np.arange(128)[:, None]
            qloc = np.arange(SC)[None, :]
            m = np.where(qloc - 128 * delta >= kloc, 2.0, 1.0)
        else:
            m = np.ones((128, SC))
        cst[:, CST_MASK + delta * SC: CST_MASK + (delta + 1) * SC] = m
    # per-pack per-partition biases: partition p of pack pk is d = pk*128+p
    cst[:, CST_BQ:CST_BQ + 2] = bq4.reshape(2, 128).T
    cst[:, CST_BK:CST_BK + 2] = bk4.reshape(2, 128).T
    # bv in free-dim layout [4*64], broadcast along partitions
    cst[:, CST_BV:CST_BV + HPC * PD] = np.broadcast_to(
        bv4, (128, HPC * PD))
    cst[:, CST_LOG2] = LOG2
    cst[:, CST_ZERO] = 0.0
    return cst


def kernel(**inputs) -> np.ndarray:
    q = np.asarray(inputs["q"], np.float32)
    k = np.asarray(inputs["k"], np.float32)
    v = np.asarray(inputs["v"], np.float32)
    Wq = np.asarray(inputs["Wq"], np.float32)
    Wk = np.asarray(inputs["Wk"], np.float32)
    Wv = np.asarray(inputs["Wv"], np.float32)
    Wo = np.asarray(inputs["Wo"], np.float32)
    bq = np.asarray(inputs["bq"], np.float32)
    bk = np.asarray(inputs["bk"], np.float32)
    bv = np.asarray(inputs["bv"], np.float32)
    bo = np.asarray(inputs["bo"], np.float32)
    causal = bool(np.asarray(inputs["use_causal_mask"]).item())

    nc = _get_program(causal)

    qTb = [np.ascontiguousarray(q[b].T) for b in range(B)]
    kTb = [np.ascontiguousarray(k[b].T) for b in range(B)]
    vTb = [np.ascontiguousarray(v[b].T) for b in range(B)]

    in_maps = []
    for c in range(NCORES):
        b, hg = divmod(c, NCORES // B)
        cols = slice(hg * HPC * PD, (hg + 1) * HPC * PD)
        in_maps.append({
            "qT": qTb[b],
            "kT": kTb[b],
            "vT": vTb[b],
            "wq": np.ascontiguousarray(Wq[:, cols]),
            "wk": np.ascontiguousarray(Wk[:, cols]),
            "wv": np.ascontiguousarray(Wv[:, cols]),
            "wo": np.ascontiguousarray(Wo[cols, :]),
            "cst": _make_cst(bq[cols], bk[cols], bv[cols], causal),
        })

    res = run_bass_kernel_spmd(nc, in_maps, list(range(NCORES)))

    out = np.empty((B, S, D), np.float32)
    ncb = NCORES // B
    for b in range(B):
        acc = res.results[b * ncb]["out"].copy()
        for c in range(b * ncb + 1, (b + 1) * ncb):
            acc += res.results[c]["out"]
        out[b] = acc + bo
    return out
